# revision 25
# baseline (speedup 1.0000x reference)
"""Trainium2 Bass kernel for nn_Attention (B=1, S=2048, D=4096, H=32, KVH=8).

Sharding: tensor-parallel over heads across 8 cores (4 Q heads + 1 KV head
per core). Each core computes QKV projections for its heads from the full
x^T, applies RoPE, runs causal flash attention in transposed layout, then a
per-head AllToAll redistributes attention outputs so each core computes the
final out-projection rows for its 1/8 of the sequence against the full wo.
The host concatenates the per-core row blocks.

Matmuls run in bf16; accumulation and softmax bookkeeping stay fp32 in PSUM.
The chip settles at a 13/16 PE clock under sustained matmul load, so streamed
PE columns are the binding resource. Main steps taken to stay at that
roofline:
- softmax rowsums are computed per GROUP of up to 4 key-chunks: the DVE
  pre-adds exp chunks (bf16) and one ones-matmul streams the group sum, so
  the PE streams ~1/3 of the rowsum columns the naive per-chunk form needs;
- P1 emits outputs in order [q0,q1,k,q2,q3,v] over a 4-deep projection-PSUM
  rotation (v last: no RoPE, so the vector queue is drained when P2's
  mask/pair-sum work arrives), and P2's PSUM pools are created in order
  (sc, o, rs) so the first score matmuls land on banks whose last readers
  finished long before - the PE never drains at the P1->P2 boundary
  (which would also drop the HAM clock to 4/8 for ~10us);
- startup DMAs are issued need-order with fine first slices (first matmul
  at ~8us instead of ~21us);
- a tiny warmup AllToAll at kernel start absorbs the ~50us first-collective
  firmware cost; per-head a2a's then hide behind attention;
- attention PV/rowsum matmuls skip fully-masked query columns of diagonal
  tiles; a cross-block software pipeline (PV trails scores by 2 pairs)
  keeps the PE dense through block and head transitions;
- wo prefetch DMA issues ride the GpSimd queue gated on a mid-P1 KT slice,
  so the 12.6MB streams during P1's idle-HBM window instead of colliding
  with the serialized a2a chain (which stretched h0's a2a from ~14us to
  ~49us); redistributed piece loads for heads 0-2 also ride GpSimd right
  after each a2a completes;
- wo blocks are staged as head-group halves (heads 01 / heads 23) with 4+4
  rotation slots so block 4+ loads start as soon as the h01 wave finishes
  with block 0, removing the wo-slot WAR stall mid-P3;
- phase 3 runs a head-wave schedule (h0/h1 for 4 no-blocks, then h2, then
  h3) so each head's a2a lands before its wave starts.
"""

import sys

for _p in ("/opt/trn_rl_repo",):
    if _p not in sys.path:
        sys.path.insert(0, _p)

from contextlib import ExitStack
from math import sqrt

import numpy as np

import concourse.bass as bass
import concourse.tile as tile
from concourse import bacc, mybir
from concourse.masks import make_identity

F32 = mybir.dt.float32
F32R = mybir.dt.float32r
BF16 = mybir.dt.bfloat16

FULL_CFG = dict(S=2048, D=4096, H=32, KVH=8, HD=128, NB=512, n_cores=8,
                dt="bf16", pre_n=3)


def build_bass(cfg):
    """Build the SPMD per-core Bass program. Same program on every core; all
    per-core differences live in the input data."""
    S, D, H, KVH, HD = cfg["S"], cfg["D"], cfg["H"], cfg["KVH"], cfg["HD"]
    NB, NC = cfg["NB"], cfg["n_cores"]
    MM = BF16 if cfg.get("dt", "bf16") == "bf16" else F32R
    HPC = H // NC          # q heads per core
    KC = D // 128          # contraction chunks for projections
    NSB = S // NB          # seq blocks
    NKJ = NB // 128        # kj 128-blocks per seq block
    R = S // NC            # output rows per core
    MT = R // 128          # output row tiles
    NO = D // 512          # out-proj column blocks
    PRE_N = min(cfg.get("pre_n", 0), NO)
    scale = 1.0 / sqrt(HD)

    assert NB % R == 0 or R % NB == 0
    PIECES = max(1, NB // R)   # a2a pieces per (head, seq block)

    nc = bacc.Bacc(
        "TRN2",
        target_bir_lowering=False,
        debug=False,
        enable_asserts=False,
        num_devices=NC,
    )

    # pre-blocked host layouts: big DMAs read contiguous per-partition lines
    xT = nc.dram_tensor("xT", [NSB, 128, KC, NB], MM, kind="ExternalInput").ap()
    wq = nc.dram_tensor("wq", [HPC, 128, KC, HD], MM, kind="ExternalInput").ap()
    wk = nc.dram_tensor("wk", [128, KC, HD], MM, kind="ExternalInput").ap()
    wv = nc.dram_tensor("wv", [128, KC, HD], MM, kind="ExternalInput").ap()
    # wo blocked by head-group halves: dim2 = 0 -> global heads i*4+{0,1}
    # (cols i*2+h), dim2 = 1 -> heads i*4+{2,3}
    wo = nc.dram_tensor("wo", [NO, 128, 2, H // 2, 512], MM,
                        kind="ExternalInput").ap()
    CS = BF16 if MM == BF16 else F32
    cosT = nc.dram_tensor("cosT", [HD // 2, S], CS, kind="ExternalInput").ap()
    sinT = nc.dram_tensor("sinT", [HD // 2, S], CS, kind="ExternalInput").ap()
    out = nc.dram_tensor("out", [R, D], F32, kind="ExternalOutput").ap()

    with tile.TileContext(nc) as tc, ExitStack() as ctx:
        dram = ctx.enter_context(tc.tile_pool(name="dram", bufs=1, space="DRAM"))
        a2a_in = [dram.tile([NC, 128, R], MM, tag=f"ain{h}", name=f"ain{h}")
                  for h in range(HPC)]
        a2a_out = [dram.tile([NC, 128, R], MM, tag=f"aout{h}", name=f"aout{h}")
                   for h in range(HPC)]
        # tiny warmup collective: absorbs the ~50us first-collective firmware
        # cost while the PE streams QKV, so the real a2a chain runs at ~13us
        warm_i = dram.tile([NC, 64], MM, tag="warm_i", name="warm_i")
        warm_o = dram.tile([NC, 64], MM, tag="warm_o", name="warm_o")

        qkvp = ctx.enter_context(tc.tile_pool(name="qkv", bufs=1))
        QT = [qkvp.tile([128, S], MM, tag=f"qt{h}", name=f"QT{h}")
              for h in range(HPC)]
        KT = qkvp.tile([128, S], MM, tag="kt", name="KT")
        Vn = qkvp.tile([128, S], MM, tag="vn", name="Vn")
        ones = qkvp.tile([128, 128], MM, tag="ones", name="ones")
        # only the d=0 strip (128 wide) and the d=1 prefix (256 wide) are
        # ever read - allocate just those
        mask0 = qkvp.tile([128, 128], MM, tag="mask0", name="mask0")
        mask1 = qkvp.tile([128, 256], MM, tag="mask1", name="mask1")
        # init pool stays open for the whole kernel: letting it close would
        # hand its SBUF addresses to wq_sb, whose first DMAs then inherit a
        # WAR wait on the gpsimd affine_selects (~15us) - the exact weights
        # the very first matmul needs
        initp = ctx.enter_context(tc.tile_pool(name="init", bufs=1))
        ones_f = initp.tile([128, 128], F32, tag="ones_f", name="ones_f")
        nc.vector.memset(ones_f, 1.0)
        nc.vector.tensor_copy(ones, ones_f)
        # causal masks for the 4 diagonal offsets (DVE multiplies, bf16)
        onb_f = initp.tile([128, 256], F32, tag="onb_f", name="onb_f")
        onb = initp.tile([128, 256], MM, tag="onb", name="onb")
        nc.vector.memset(onb_f, 1.0)
        nc.vector.tensor_copy(onb, onb_f)
        nc.gpsimd.affine_select(
            out=mask0, in_=onb[:, :128], compare_op=mybir.AluOpType.is_ge,
            fill=0.0, base=0, pattern=[[1, 128]],
            channel_multiplier=-1,
        )
        nc.gpsimd.affine_select(
            out=mask1, in_=onb, compare_op=mybir.AluOpType.is_ge,
            fill=0.0, base=-128, pattern=[[1, 256]],
            channel_multiplier=-1,
        )
        nc.gpsimd.collective_compute(
            "AllToAll",
            mybir.AluOpType.bypass,
            replica_groups=[list(range(NC))],
            ins=[warm_i.opt()],
            outs=[warm_o.opt()],
        )

        # ============= Phase 1: QKV projection + RoPE =============
        with (
            tc.tile_pool(name="wgt", bufs=1) as wgt,
            tc.tile_pool(name="trig", bufs=1) as trig,
            tc.tile_pool(name="xtp", bufs=1) as xtp,
            tc.tile_pool(name="vt_sb", bufs=1) as vtp,
            tc.tile_pool(name="pj_ps", bufs=6, space="PSUM") as pjps,
            tc.tile_pool(name="tps", bufs=2, space="PSUM") as tps,
            tc.tile_pool(name="rope_t", bufs=4) as ropep,
        ):
            hw_ = HD // 2
            cos_sb = trig.tile([hw_, S], CS, tag="cos", name="cos_sb")
            sin_sb = trig.tile([hw_, S], CS, tag="sin", name="sin_sb")
            ident = trig.tile([128, 128], MM, tag="ident", name="ident")
            make_identity(nc, ident)

            wq_sb = wgt.tile([128, HPC, KC, HD], MM, tag="wq", name="wq_sb")
            wk_sb = wgt.tile([128, KC, HD], MM, tag="wk", name="wk_sb")
            wv_sb = wgt.tile([128, KC, HD], MM, tag="wv", name="wv_sb")
            # need-ordered weight stream: block 0 runs as two half-
            # contraction passes over all six outputs, so pass-1 halves of
            # every panel come first, then the pass-2 halves - the first
            # ~25us of PE work then needs only ~250GB/s of delivery
            KH2 = KC // 2
            for a, b in ((0, 1), (1, 2), (2, 5), (5, 10), (10, 16)):
                nc.scalar.dma_start(out=wq_sb[:, 0, a:b], in_=wq[0][:, a:b])
            nc.scalar.dma_start(out=wq_sb[:, 1, :KH2], in_=wq[1][:, :KH2])
            nc.scalar.dma_start(out=wk_sb[:, :KH2], in_=wk[:, :KH2])
            nc.scalar.dma_start(out=wq_sb[:, 2, :KH2], in_=wq[2][:, :KH2])
            nc.scalar.dma_start(out=wq_sb[:, 3, :KH2], in_=wq[3][:, :KH2])
            nc.scalar.dma_start(out=wv_sb[:, :KH2], in_=wv[:, :KH2])
            nc.scalar.dma_start(out=wq_sb[:, 0, KH2:], in_=wq[0][:, KH2:])
            nc.scalar.dma_start(out=wq_sb[:, 1, KH2:], in_=wq[1][:, KH2:])
            nc.scalar.dma_start(out=wk_sb[:, KH2:], in_=wk[:, KH2:])
            nc.scalar.dma_start(out=cos_sb, in_=cosT)
            nc.scalar.dma_start(out=sin_sb, in_=sinT)
            nc.scalar.dma_start(out=wq_sb[:, 2, KH2:], in_=wq[2][:, KH2:])
            nc.scalar.dma_start(out=wq_sb[:, 3, KH2:], in_=wq[3][:, KH2:])
            nc.scalar.dma_start(out=wv_sb[:, KH2:], in_=wv[:, KH2:])

            VT_sb = vtp.tile([128, S], MM, tag="vt", name="VT_sb")

            # output order: q0 first (wq0 heads the weight stream), v
            # LAST: v has no RoPE, so the vector queue is already drained
            # when P2's first masks/pair-sums arrive, and q3's RoPE (the
            # last vector chain) retires ~4us before P1 ends
            outs = [(lambda k: wq_sb[:, 0, k, :], "q", 0),
                    (lambda k: wq_sb[:, 1, k, :], "q", 1),
                    (lambda k: wk_sb[:, k, :], "k", 0),
                    (lambda k: wq_sb[:, 2, k, :], "q", 2),
                    (lambda k: wq_sb[:, 3, k, :], "q", 3),
                    (lambda k: wv_sb[:, k, :], "v", 0)]

            XSP0 = [(0, 1), (1, 1), (2, 2), (4, 4), (8, 8), (16, 8), (24, 8)]
            XSP = [(0, 8), (8, 8), (16, 8), (24, 8)]
            # w8 bufs=6: block 1 may stage only 3 of its 4 tiles early, so
            # its x stream doesn't steal HBM bandwidth from block 0's
            # weights during the startup ramp
            XBUFS = {1: 2, 2: 1, 4: 1, 8: 6}
            for n in range(NSB):
                nsl = slice(n * NB, (n + 1) * NB)
                xh = []
                for s0, w in (XSP0 if n == 0 else XSP):
                    xt = xtp.tile([128, w, NB], MM, tag=f"xw{w}", name="xb",
                                  bufs=XBUFS[w])
                    if n == 0 and s0 == 8:
                        nc.sync.dma_start(out=xt[:, :4],
                                          in_=xT[n, :, s0:s0 + 4, :])
                        nc.sync.dma_start(out=xt[:, 4:],
                                          in_=xT[n, :, s0 + 4:s0 + w, :])
                    else:
                        nc.sync.dma_start(out=xt, in_=xT[n, :, s0:s0 + w, :])
                    xh.append((s0, w, xt))

                def xrhs(k):
                    for s0, w, xt in xh:
                        if s0 <= k < s0 + w:
                            return xt[:, k - s0, :]

                # output-major over a 6-deep PSUM rotation (one fixed bank
                # per output). Block 0 runs two half-contraction passes:
                # the first 2.3MB of x+weights covers ~25us of PE work, so
                # the startup ramp no longer outruns HBM delivery
                if n == 0:
                    plan = []
                    for wsel, kind, h in outs:
                        ps = pjps.tile([128, NB], F32, tag="pj", name="ps")
                        for k in range(KH2):
                            nc.tensor.matmul(ps, lhsT=wsel(k), rhs=xrhs(k),
                                             start=(k == 0), stop=False)
                        plan.append(((wsel, kind, h), ps, KH2))
                else:
                    plan = [(o, None, 0) for o in outs]
                for (wsel, kind, h), ps0, k0 in plan:
                    ps = ps0 if ps0 is not None else pjps.tile(
                        [128, NB], F32, tag="pj", name="ps")
                    for k in range(k0, KC):
                        nc.tensor.matmul(
                            ps, lhsT=wsel(k), rhs=xrhs(k),
                            start=(k == 0), stop=(k == KC - 1),
                        )
                    if kind == "v":
                        if n == NSB - 1:
                            for t4 in range(4):
                                csl = slice(n * NB + t4 * 128,
                                            n * NB + (t4 + 1) * 128)
                                nc.scalar.copy(VT_sb[:, csl],
                                               ps[:, t4 * 128:(t4 + 1) * 128])
                        else:
                            nc.scalar.copy(VT_sb[:, nsl], ps)
                        for t in range(n * NB // 128, (n + 1) * NB // 128):
                            tsl = slice(t * 128, (t + 1) * 128)
                            pst = tps.tile([128, 128], MM, tag="tp",
                                           name="pst")
                            nc.tensor.transpose(pst, VT_sb[:, tsl], ident)
                            if n == NSB - 1:
                                # vector is idle at P1's end (q3's RoPE
                                # retired early); halving the scalar chain
                                # shortens the P1->P2 handoff
                                nc.vector.tensor_copy(Vn[:, tsl], pst)
                            else:
                                nc.scalar.copy(Vn[:, tsl], pst)
                    else:
                        dst = KT if kind == "k" else QT[h]
                        E, O = ps[0:hw_, :], ps[hw_:2 * hw_, :]
                        c, s = cos_sb[:, nsl], sin_sb[:, nsl]
                        t1 = ropep.tile([hw_, NB], F32, tag="rt", name="t1")
                        t2 = ropep.tile([hw_, NB], F32, tag="rt", name="t2")
                        nc.vector.tensor_mul(t1, E, c)
                        nc.vector.tensor_mul(t2, O, s)
                        nc.vector.tensor_sub(dst[0:hw_, nsl], t1, t2)
                        t3 = ropep.tile([hw_, NB], F32, tag="rt", name="t3")
                        t4 = ropep.tile([hw_, NB], F32, tag="rt", name="t4")
                        nc.vector.tensor_mul(t3, E, s)
                        nc.vector.tensor_mul(t4, O, c)
                        nc.vector.tensor_add(dst[hw_:, nsl], t3, t4)


        # P3 SBUF pools open before P2 so their DMAs can prefetch during it
        pcp = ctx.enter_context(tc.tile_pool(name="pc_sb", bufs=1))
        wop = ctx.enter_context(tc.tile_pool(name="wo_sb", bufs=1))
        obp = ctx.enter_context(tc.tile_pool(name="ob_sb", bufs=2))
        piece = {}

        wo_pre = {}

        def load_wob_half(no, half, eng):
            wt = wop.tile([128, H // 2, 512], MM, tag=f"wob{half}",
                          name=f"wob{half}", bufs=4)
            eng.dma_start(out=wt, in_=wo[no, :, half])
            wo_pre[(no, half)] = wt

        def get_wob(no):
            # post-wave blocks: issue on the gpsimd queue (idle in P3; the
            # slot-WAR wait blocks only later wo issues, not compute queues)
            if (no, 0) not in wo_pre:
                load_wob_half(no, 0, nc.gpsimd)
            if (no, 1) not in wo_pre:
                load_wob_half(no, 1, nc.gpsimd)
            return wo_pre[(no, 0)], wo_pre[(no, 1)]



        # wo prefetch: gate the issues on a KT slice written mid-P1 (block
        # 1's k RoPE, ~127us) via a tiny gpsimd copy, then issue on the
        # gpsimd queue. The 12.6MB then streams during P1's idle-HBM window
        # - far from both the startup ramp and the serialized a2a chain
        # (overlapping the chain stretched h0's a2a from ~14us to ~49us)
        gate = initp.tile([128, 4], MM, tag="wogate", name="wogate")
        nc.gpsimd.tensor_copy(gate, KT[:, NB * 2 - 4:NB * 2])
        for i in range(PRE_N):
            load_wob_half(i, 0, nc.gpsimd)
            load_wob_half(i, 1, nc.gpsimd)

        # ============= Phase 2: causal flash attention =============
        # PSUM pool order (sc, o, rs): sc lands on banks 0-3 whose last P1
        # readers (q-RoPEs / v copy) all finish before or right at P1's
        # end with the [q0,k,q1,q2,q3,v] output order; o gets the tps
        # banks (Vn copies), rs is virgin - no WAR stall at the transition
        with (
            tc.tile_pool(name="sc_ps", bufs=2, space="PSUM") as scps,
            tc.tile_pool(name="o_ps", bufs=2, space="PSUM") as ops_,
            tc.tile_pool(name="rs_ps", bufs=2, space="PSUM") as rsps,
            tc.tile_pool(name="exp_sb", bufs=4) as exps,
            tc.tile_pool(name="att_sb", bufs=5) as atts,
            tc.tile_pool(name="pr_sb", bufs=1) as prp,
        ):
            # unified cross-block software pipeline: PV pairs trail the score
            # stream by 2 pairs ACROSS block boundaries, and each block's
            # normalize + piece stores are emitted once its last PV pops -
            # so the PE never drains at a block boundary
            inflight = []

            def finish_block(b):
                rcp = atts.tile([128, NB], F32, tag="rcp", name="rcp")
                nc.vector.reciprocal_approx_fast(rcp, b["rs_ps"])
                o_sb = atts.tile([128, NB], MM, tag="osb", name="o_sb")
                nc.vector.tensor_mul(o_sb, b["o_ps"], rcp)
                hh, nn = b["h"], b["n"]
                for jj in range(PIECES):
                    piece_idx = nn * PIECES + jj
                    nc.sync.dma_start(
                        out=a2a_in[hh][piece_idx],
                        in_=o_sb[:, jj * R:(jj + 1) * R],
                    )
                if nn == NSB - 1:
                    nc.gpsimd.collective_compute(
                        "AllToAll",
                        mybir.AluOpType.bypass,
                        replica_groups=[list(range(NC))],
                        ins=[a2a_in[hh].opt()],
                        outs=[a2a_out[hh].opt()],
                    )
                    if hh < HPC - 1:
                        # pull heads 0-2's redistributed pieces in on the
                        # gpsimd queue right behind their a2a: ready well
                        # before P3's waves, and the a2a-completion wait
                        # only blocks later gpsimd issues
                        for i in range(NC):
                            t = pcp.tile([128, R], MM, tag=f"pc{hh}_{i}",
                                         name=f"pc{hh}_{i}")
                            nc.gpsimd.dma_start(out=t, in_=a2a_out[hh][i])
                            piece[(hh, i)] = t

            def emit_pv(entry):
                pair, b, jp = entry
                n = b["n"]
                # grouped rowsum first: one ones-matmul per DVE-pre-summed
                # group of up to 4 chunks (quads of off-diagonal pairs +
                # the lone (d0,d1) pair); diagonal d2/d3 get narrow ones
                g = b["grp"].pop(jp, None)
                if g is not None:
                    nc.tensor.matmul(b["rs_ps"], lhsT=ones, rhs=g,
                                     start=b["rs_first"], stop=False)
                    b["rs_first"] = False
                for (j, pex) in pair:
                    d = j - n * NKJ
                    if d >= 2:
                        qs = d * 128
                        nc.tensor.matmul(b["rs_ps"][:, qs:], lhsT=ones,
                                         rhs=pex[:, qs:],
                                         start=False, stop=(j == b["last_j"]))
                for (j, pex) in pair:
                    psl = slice(j * 128, (j + 1) * 128)
                    d = j - n * NKJ
                    last = j == b["last_j"]
                    if d > 0:
                        # diagonal tile: query cols < d*128 are fully masked
                        # (pex is 0 there) - skip streaming them
                        qs = d * 128
                        nc.tensor.matmul(b["o_ps"][:, qs:],
                                         lhsT=Vn[:, psl], rhs=pex[:, qs:],
                                         start=False, stop=last)
                    else:
                        nc.tensor.matmul(b["o_ps"], lhsT=Vn[:, psl], rhs=pex,
                                         start=(j == 0), stop=last)
                b["npv"] -= 1
                if b["npv"] == 0:
                    finish_block(b)

            def pump(maxlag):
                while len(inflight) > maxlag:
                    emit_pv(inflight.pop(0))

            for h in range(HPC):
                for n in range(NSB):
                    nsl = slice(n * NB, (n + 1) * NB)
                    nkj = (n + 1) * NKJ
                    b = dict(
                        o_ps=ops_.tile([128, NB], F32, tag="o", name="o_ps"),
                        rs_ps=rsps.tile([128, NB], F32, tag="rs",
                                        name="rs_ps"),
                        h=h, n=n, last_j=nkj - 1, npv=nkj // 2,
                        grp={}, pr={}, rs_first=True,
                    )
                    q_rhs = QT[h][:, nsl]

                    for jp in range(nkj // 2):
                        sc = scps.tile([128, 2, NB], F32, tag="sc",
                                       name="sc")
                        ex = exps.tile([128, 2, NB], MM, tag="ex", name="ex")
                        pair = []
                        for half in range(2):
                            j = 2 * jp + half
                            jsl = slice(j * 128, (j + 1) * 128)
                            d = j - n * NKJ
                            if d >= 2:
                                # deep-diagonal tile: query cols < d*128 are
                                # fully masked downstream - skip them in the
                                # score stream too. d==1 streams full so the
                                # (0,1) pair keeps its single fused exp (the
                                # ACT op overhead costs more than 128 PE
                                # columns)
                                qs = d * 128
                                nc.tensor.matmul(sc[:, half, qs:],
                                                 lhsT=KT[:, jsl],
                                                 rhs=q_rhs[:, qs:],
                                                 start=True, stop=True)
                            else:
                                nc.tensor.matmul(sc[:, half], lhsT=KT[:, jsl],
                                                 rhs=q_rhs, start=True,
                                                 stop=True)
                            pair.append((j, ex[:, half]))
                        pump(3)
                        if 2 * jp + 1 - n * NKJ <= 1:
                            nc.scalar.activation(
                                ex.rearrange("p a b -> p (a b)"),
                                sc.rearrange("p a b -> p (a b)"),
                                mybir.ActivationFunctionType.Exp, scale=scale,
                            )
                        else:
                            # sliced exps so the dead region of sc (stale
                            # PSUM) is never read; the dead region of ex is
                            # never read either (PV/rowsum skip it)
                            for half in range(2):
                                d = 2 * jp + half - n * NKJ
                                qs = d * 128
                                nc.scalar.activation(
                                    ex[:, half, qs:], sc[:, half, qs:],
                                    mybir.ActivationFunctionType.Exp,
                                    scale=scale,
                                )
                        for half in range(2):
                            j = 2 * jp + half
                            d = j - n * NKJ
                            if d == 1:
                                # d=1 streamed full: zero cols [0,128) and
                                # partial-mask [128,256) in one multiply
                                nc.vector.tensor_mul(ex[:, half, :256],
                                                     ex[:, half, :256],
                                                     mask1)
                            elif d >= 0:
                                # only the 128-wide strip crossing the
                                # diagonal is partial; it matches masks[0]'s
                                # leading strip for every offset d
                                qs = d * 128
                                nc.vector.tensor_mul(
                                    ex[:, half, qs:qs + 128],
                                    ex[:, half, qs:qs + 128],
                                    mask0)
                        if jp <= 2 * n:
                            # groupable pair (both chunks d<=1, masked cols
                            # are exact zeros): DVE pair-sum in bf16; the
                            # bf16 rounding lands on the fp32 denominator at
                            # ~0.1% - well inside budget
                            pr = prp.tile([128, NB], MM, tag="pr", name="pr",
                                          bufs=4)
                            nc.vector.tensor_add(pr, ex[:, 0, :],
                                                 ex[:, 1, :])
                            b["pr"][jp] = pr
                            if jp % 2 == 1:
                                qd = prp.tile([128, NB], MM, tag="qd",
                                              name="qd", bufs=3)
                                nc.vector.tensor_add(qd, b["pr"][jp - 1],
                                                     b["pr"][jp])
                                b["grp"][jp] = qd
                            elif jp == 2 * n:
                                b["grp"][jp] = pr
                        inflight.append((pair, b, jp))
            pump(0)
            # the P3 PSUM pool-open barrier waits for the last block's
            # normalize; keep the PE minimally busy across that ~1.5us so
            # the HAM doesn't drop the clock to 4/8 for 3.4us
            dmy = scps.tile([128, 2, NB], F32, tag="sc", name="dmy")
            for _ in range(8):
                nc.tensor.matmul(dmy[:, 0, :128], lhsT=ones,
                                 rhs=ones, start=True, stop=True)

            # head 3's pieces load on the sync queue after its stores (the
            # queue is idle there; earlier heads already loaded via gpsimd)
            for i in range(NC):
                t = pcp.tile([128, R], MM, tag=f"pc{HPC - 1}_{i}",
                             name=f"pc{HPC - 1}_{i}")
                nc.sync.dma_start(out=t, in_=a2a_out[HPC - 1][i])
                piece[(HPC - 1, i)] = t


        # ============= Phase 3: out-projection =============
        with tc.tile_pool(name="op_ps", bufs=2, space="PSUM") as opps:

            def emit_mms(pso, wob2, heads, first):
                wA, wB = wob2
                for m in range(MT):
                    f = first
                    for h in heads:
                        for i in range(NC):
                            wt = wA if h < 2 else wB
                            col = i * 2 + (h % 2)
                            last = (h == HPC - 1) and (i == NC - 1)
                            nc.tensor.matmul(
                                pso[m],
                                lhsT=piece[(h, i)][:, m * 128:(m + 1) * 128],
                                rhs=wt[:, col, :], start=f, stop=last,
                            )
                            f = False
                return False

            def drain(pso, no):
                osl = slice(no * 512, (no + 1) * 512)
                for m in range(MT):
                    ob = obp.tile([128, 512], F32, tag="ob", name="ob")
                    nc.vector.tensor_copy(ob, pso[m])
                    nc.sync.dma_start(
                        out=out[m * 128:(m + 1) * 128, osl], in_=ob)

            def alloc_pso():
                return [opps.tile([128, 512], F32, tag=f"po{m}",
                                  name=f"pso{m}", bufs=max(2, DEFER))
                        for m in range(MT)]

            def emit_mms_m(pso_m, wob2, heads, m, first):
                wA, wB = wob2
                f = first
                for h in heads:
                    for i in range(NC):
                        wt = wA if h < 2 else wB
                        nc.tensor.matmul(
                            pso_m,
                            lhsT=piece[(h, i)][:, m * 128:(m + 1) * 128],
                            rhs=wt[:, i * 2 + (h % 2), :], start=f,
                            stop=(h == HPC - 1) and (i == NC - 1))
                        f = False

            # wave schedule over the first DEFER no-blocks: heads {0,1}
            # (pair-0 a2a, done mid-attention) first, then h2 and h3 whose
            # pair-1 a2a lands while the h01 wave streams
            DEFER = min(NO, 4) if HPC > 1 else 0
            state = {}
            for no in range(DEFER):
                wob2 = get_wob(no)
                if no == 0:
                    # po0's slab (first allocation) = P2's sc banks, idle
                    # since the last exps; po1 (the o/rs banks, whose last
                    # readers are the final block's normalize) is allocated
                    # only after m=0's matmuls so its WAR wait sits mid-wave
                    # in the PE FIFO instead of blocking the wave start
                    p0 = opps.tile([128, 512], F32, tag="po0", name="pso0",
                                   bufs=max(2, DEFER))
                    emit_mms_m(p0, wob2, [0, 1], 0, True)
                    p1 = opps.tile([128, 512], F32, tag="po1", name="pso1",
                                   bufs=max(2, DEFER))
                    emit_mms_m(p1, wob2, [0, 1], 1, True)
                    pso = [p0, p1]
                else:
                    pso = alloc_pso()
                    emit_mms(pso, wob2, [0, 1], True)
                state[no] = (pso, wob2)
            for no in range(DEFER):
                pso, wob2 = state[no]
                emit_mms(pso, wob2, [2], False)
            for no in range(DEFER):
                pso, wob2 = state[no]
                emit_mms(pso, wob2, [3], False)
                drain(pso, no)
            for no in range(DEFER, NO - 1):
                wob2 = get_wob(no)
                pso = alloc_pso()
                emit_mms(pso, wob2, range(HPC), True)
                drain(pso, no)

            # final block: stream m=0 fully and drain it, then m=1 in two
            # column halves so the last copy+store tail is ~256 cols
            # (~1.8us) instead of 512 (~3.9us)
            no = NO - 1
            wA, wB = get_wob(no)
            pso = alloc_pso()
            f = True
            for h in range(HPC):
                for i in range(NC):
                    wt = wA if h < 2 else wB
                    nc.tensor.matmul(
                        pso[0], lhsT=piece[(h, i)][:, 0:128],
                        rhs=wt[:, i * 2 + (h % 2), :], start=f,
                        stop=(h == HPC - 1) and (i == NC - 1))
                    f = False
            ob = obp.tile([128, 512], F32, tag="ob", name="ob")
            nc.vector.tensor_copy(ob, pso[0])
            nc.sync.dma_start(out=out[0:128, no * 512:(no + 1) * 512], in_=ob)
            for cs in (slice(0, 256), slice(256, 512)):
                f = True
                for h in range(HPC):
                    for i in range(NC):
                        wt = wA if h < 2 else wB
                        nc.tensor.matmul(
                            pso[1][:, cs],
                            lhsT=piece[(h, i)][:, 128:256],
                            rhs=wt[:, i * 2 + (h % 2), cs], start=f,
                            stop=(h == HPC - 1) and (i == NC - 1))
                        f = False
                obh = obp.tile([128, 256], F32, tag="obh", name="obh",
                               bufs=2)
                nc.vector.tensor_copy(obh, pso[1][:, cs])
                nc.sync.dma_start(
                    out=out[128:256,
                            no * 512 + cs.start:no * 512 + cs.stop],
                    in_=obh)

    nc.compile()
    return nc


def prep_inputs(cfg, x, wq, wk, wv, wo, freqs_cos, freqs_sin):
    """Host-side sharding/layout prep. Returns list of per-core input dicts."""
    S, D, H, KVH, HD, NC = (cfg["S"], cfg["D"], cfg["H"], cfg["KVH"], cfg["HD"],
                            cfg["n_cores"])
    HPC = H // NC
    if cfg.get("dt", "bf16") == "bf16":
        import ml_dtypes
        mdt = ml_dtypes.bfloat16
    else:
        mdt = np.float32
    x = np.asarray(x, np.float32).reshape(S, D)
    wq = np.asarray(wq, np.float32)
    wk = np.asarray(wk, np.float32)
    wv = np.asarray(wv, np.float32)
    wo = np.asarray(wo, np.float32)
    cos = np.asarray(freqs_cos, np.float32)
    sin = np.asarray(freqs_sin, np.float32)

    NSB, NB = S // cfg["NB"], cfg["NB"]
    KC = D // 128
    NO = D // 512
    # x blocked: xp[n, p, k, s] = x[n*NB+s, k*128+p]
    xp = np.ascontiguousarray(
        x.reshape(NSB, NB, KC, 128).transpose(0, 3, 2, 1)).astype(mdt)
    cosT = np.ascontiguousarray(cos.T).astype(mdt)      # [HD/2, S]
    sinT = np.ascontiguousarray(sin.T).astype(mdt)
    # wo blocked: woc[no, p, g, m] = wo[g*128+p, no*512+m], then regrouped
    # into head-group halves: [no, p, a, i*2+hh, m] with g = i*4 + a*2 + hh
    wo_c = np.ascontiguousarray(
        wo.reshape(H, 128, NO, 512).transpose(2, 1, 0, 3))
    G = np.array([[(j // 2) * 4 + a * 2 + (j % 2) for j in range(H // 2)]
                  for a in range(2)])
    wo_c2 = np.ascontiguousarray(wo_c[:, :, G, :]).astype(mdt)

    # de-interleave rope pairs: new col i <- 2i, new col i+HD/2 <- 2i+1
    idx = np.concatenate([np.arange(0, HD, 2), np.arange(1, HD, 2)])
    wq_p = wq.reshape(D, H, HD)[:, :, idx]
    wk_p = wk.reshape(D, KVH, HD)[:, :, idx]
    wv_r = wv.reshape(D, KVH, HD)

    def wblock(w):
        # [D, M] -> [128, KC, M] with wb[p, k, m] = w[k*128+p, m]
        M = w.shape[1]
        return np.ascontiguousarray(
            w.reshape(KC, 128, M).transpose(1, 0, 2)).astype(mdt)

    in_maps = []
    for c in range(NC):
        kv = c * KVH // NC
        in_maps.append(dict(
            xT=xp,
            wq=np.stack([wblock(np.ascontiguousarray(wq_p[:, c * HPC + h]))
                         for h in range(HPC)]),
            wk=wblock(np.ascontiguousarray(wk_p[:, kv])),
            wv=wblock(np.ascontiguousarray(wv_r[:, kv])),
            wo=wo_c2,
            cosT=cosT,
            sinT=sinT,
        ))
    return in_maps


_CACHED = {}


def _get_nc(cfg_key=None):
    if "nc" not in _CACHED:
        _CACHED["nc"] = build_bass(FULL_CFG)
    return _CACHED["nc"]


def run_spmd(x, wq, wk, wv, wo, freqs_cos, freqs_sin, **spmd_kwargs):
    """Build (cached), run on 8 cores, return (full_output, BassKernelResults)."""
    from concourse.bass_utils import run_bass_kernel_spmd

    cfg = FULL_CFG
    NC = cfg["n_cores"]
    in_maps = prep_inputs(cfg, x, wq, wk, wv, wo, freqs_cos, freqs_sin)
    nc = _get_nc()
    res = run_bass_kernel_spmd(nc, in_maps, list(range(NC)), **spmd_kwargs)
    parts = [res.results[c]["out"] for c in range(NC)]
    full = np.concatenate(parts, axis=0)
    return full.reshape(1, cfg["S"], cfg["D"]).astype(np.float32), res


def kernel(x, wq, wk, wv, wo, freqs_cos, freqs_sin):
    out, _ = run_spmd(x, wq, wk, wv, wo, freqs_cos, freqs_sin)
    return out
